# revision 1
# baseline (speedup 1.0000x reference)
"""Trainium2 Bass kernel for nn_Conv3DSynthesisLayer.

Computes, per sample b (one NeuronCore each, data-parallel over batch B=8):
  styles = w[b] @ (affine_weight / sqrt(512)).T + affine_bias        [Cin]
  wmod   = weight * styles[None,:,None..] ; demod by rsqrt(sumsq)    [Cout,Cin,3,3,3]
  out    = lrelu(conv3d(x[b], wmod, pad=1) + bias) * sqrt(2)         [Cout,32,32,32]

Implementation notes:
  - Conv is 27 shifted fp32r matmuls (K=Cin=128 on partitions) accumulated in
    PSUM per output d-slice, over an h/w zero-padded x laid out per-slice as
    [128, 34*34] in SBUF.  D-boundary taps are skipped (no d padding).
  - x is DMA'd contiguously into an f32 staging tile, then placed (and rounded
    to f32r) into the padded layout by the DVE; halos are zeroed by DVE copies
    from a zero tile (DMA cannot produce f32r-rounded data, DVE can).
  - Demodulation (per-Cout scale) and the lrelu(.)*sqrt(2) epilogue are fused
    into one ScalarE Prelu op per PSUM bank: out = prelu(psum*scale + bias*g).
    (lrelu is positively homogeneous: g*lrelu(z) == lrelu(g*z).)
  - Weight transposes (Cout-major -> Cin-major for matmul lhsT) run on the PE
    via identity transpose; styles modulation is applied by the DVE on the
    PSUM->SBUF copy-back.
  - Main loop is taps-outer over groups of GD d-slices so matmuls that share
    a stationary operand are adjacent.
"""
import sys

sys.path.insert(0, "/opt/trn_rl_repo")

import numpy as np
from contextlib import ExitStack

import concourse.mybir as mybir
import concourse.tile as tile
from concourse import bacc
from concourse.masks import make_identity
from concourse import bass_utils as _bass_utils
from concourse.bass_utils import run_bass_kernel_spmd

# Enable walrus's LDWEIGHTS dedup so consecutive matmuls sharing a stationary
# operand skip the redundant ~191ns weight reload (fp32r matmuls are emitted
# self-loading; the default pipeline pins --enable-ldw-opt=false).
_LDW_OPT = True
if not getattr(_bass_utils, "_ldw_opt_patched", False):
    _orig_run_command = _bass_utils.run_command

    def _run_command_ldw(argv, **kw):
        if _LDW_OPT and isinstance(argv, (list, tuple)):
            argv = ["--enable-ldw-opt=true" if a == "--enable-ldw-opt=false" else a
                    for a in argv]
        return _orig_run_command(argv, **kw)

    _bass_utils.run_command = _run_command_ldw
    _bass_utils._ldw_opt_patched = True

F32 = mybir.dt.float32
F32R = mybir.dt.float32r
AF = mybir.ActivationFunctionType

B, CIN, COUT, R = 8, 128, 128, 32
W_DIM = 512
NTAPS = 27
RP = R + 2  # 34: h/w padded
GAIN = float(np.sqrt(2.0).astype(np.float32))
SLOPE = 0.2
EPS = 1e-8
DBLK = 4  # d-slices per x block
NBLK = R // DBLK
GD = 2  # d-slices per psum group (taps-outer)
NCORES = 8

_cache = {}


def _build():
    nc = bacc.Bacc("TRN2", target_bir_lowering=False, debug=False, num_devices=NCORES)
    x_d = nc.dram_tensor("x", [CIN, R * R * R], F32, kind="ExternalInput").ap()
    wv_d = nc.dram_tensor("wvec", [W_DIM], F32, kind="ExternalInput").ap()
    wt_d = nc.dram_tensor("weight", [COUT, CIN * NTAPS], F32, kind="ExternalInput").ap()
    aw_d = nc.dram_tensor("aw", [CIN, W_DIM], F32, kind="ExternalInput").ap()
    ab_d = nc.dram_tensor("ab", [CIN], F32, kind="ExternalInput").ap()
    bs_d = nc.dram_tensor("bias", [COUT], F32, kind="ExternalInput").ap()
    out_d = nc.dram_tensor("out", [COUT, R * R * R], F32, kind="ExternalOutput").ap()

    ctx = ExitStack()
    with ctx:
        tc = ctx.enter_context(tile.TileContext(nc))
        singles = ctx.enter_context(tc.tile_pool(name="singles", bufs=1))
        xpool = ctx.enter_context(tc.tile_pool(name="xpool", bufs=4))
        stpool = ctx.enter_context(tc.tile_pool(name="stpool", bufs=2))
        obpool = ctx.enter_context(tc.tile_pool(name="obpool", bufs=4))

        # ---- phase A: styles, modulated transposed weights, demod scale ----
        with tc.tile_pool(name="ps_a", bufs=2, space="PSUM") as ps_a:
            ident = singles.tile([128, 128], F32)
            make_identity(nc, ident)

            aw_sb = singles.tile([128, W_DIM], F32)
            nc.sync.dma_start(out=aw_sb, in_=aw_d)
            wv_sb = singles.tile([128, 4], F32)
            nc.sync.dma_start(out=wv_sb, in_=wv_d.rearrange("(c k) -> k c", k=128))
            ab_sb = singles.tile([128, 1], F32)
            nc.sync.dma_start(out=ab_sb, in_=ab_d.rearrange("(p one) -> p one", one=1))
            bs_sb = singles.tile([128, 1], F32)
            nc.sync.dma_start(out=bs_sb, in_=bs_d.rearrange("(p one) -> p one", one=1))
            wnat = singles.tile([128, CIN * NTAPS], F32)
            nc.sync.dma_start(out=wnat, in_=wt_d)

            # affine_weight.T, chunked over the 512-dim: awt[k%128, ci] per chunk
            awt = singles.tile([128, W_DIM], F32)
            for c in range(4):
                paw = ps_a.tile([128, 128], F32, tag="paw", name=f"paw{c}")
                nc.tensor.transpose(paw, aw_sb[:, c * 128:(c + 1) * 128], ident)
                nc.vector.tensor_copy(out=awt[:, c * 128:(c + 1) * 128], in_=paw)

            # styles[ci] = sum_k aw[ci,k] w[k] / sqrt(512) + ab[ci], as [128,1]
            ps_sty = ps_a.tile([128, 1], F32, tag="ps_sty")
            for c in range(4):
                nc.tensor.matmul(ps_sty, lhsT=awt[:, c * 128:(c + 1) * 128],
                                 rhs=wv_sb[:, c:c + 1], start=(c == 0), stop=(c == 3))
            styles = singles.tile([128, 1], F32)
            nc.scalar.activation(out=styles, in_=ps_sty, func=AF.Identity,
                                 bias=ab_sb, scale=1.0 / float(np.sqrt(W_DIM)))

            # per-tap transpose [co,ci] -> [ci,co], modulate by styles[ci];
            # produce both the f32 copy (for sumsq) and the rounded f32r lhsT.
            wnat_t = wnat.rearrange("p (ci t) -> p t ci", t=NTAPS)
            w1f = singles.tile([128, NTAPS * 128], F32)
            w1r = singles.tile([128, NTAPS * 128], F32R)
            for t in range(NTAPS):
                pw = ps_a.tile([128, 128], F32, tag="paw", name=f"pw{t}")
                nc.tensor.transpose(pw, wnat_t[:, t, :], ident)
                nc.vector.tensor_scalar_mul(out=w1f[:, t * 128:(t + 1) * 128],
                                            in0=pw, scalar1=styles)
                nc.vector.tensor_copy(out=w1r[:, t * 128:(t + 1) * 128],
                                      in_=w1f[:, t * 128:(t + 1) * 128])

            # sumsq[co] = sum_{ci,t} w1f[ci, t*128+co]^2  via ones-matmuls
            sq = singles.tile([128, NTAPS * 128], F32)
            nc.vector.tensor_mul(out=sq, in0=w1f, in1=w1f)
            ones_sb = singles.tile([128, 1], F32)
            nc.vector.memset(ones_sb, 1.0)
            ps_dm = ps_a.tile([128, 1], F32, tag="ps_dm")
            for t in range(NTAPS):
                nc.tensor.matmul(ps_dm, lhsT=sq[:, t * 128:(t + 1) * 128],
                                 rhs=ones_sb, start=(t == 0), stop=(t == NTAPS - 1))
            # scale[co] = GAIN * rsqrt(sumsq+EPS) = 1/sqrt(sumsq/G^2 + EPS/G^2)
            eps_sb = singles.tile([128, 1], F32)
            nc.vector.memset(eps_sb, EPS / (GAIN * GAIN))
            sc_tmp = singles.tile([128, 1], F32)
            nc.scalar.activation(out=sc_tmp, in_=ps_dm, func=AF.Sqrt,
                                 bias=eps_sb, scale=1.0 / (GAIN * GAIN))
            scale_sb = singles.tile([128, 1], F32)
            nc.vector.reciprocal(out=scale_sb, in_=sc_tmp)
            bias_g = singles.tile([128, 1], F32)
            nc.scalar.mul(out=bias_g, in_=bs_sb, mul=GAIN)

        pspool = ctx.enter_context(tc.tile_pool(name="pspool", bufs=2, space="PSUM"))

        # ---- phase B: the conv ----
        zrow = singles.tile([128, DBLK * RP], F32)
        nc.vector.memset(zrow, 0.0)
        zview = zrow.rearrange("p (d e) -> p d e", e=RP)

        x_r = x_d.rearrange("p (d hw) -> p d hw", hw=R * R)
        xblocks = [None] * NBLK

        def load_block(blk):
            # contiguous DMA into f32 staging, then DVE place+round into the
            # h/w-padded f32r block tile (halos zeroed by DVE copies).
            stag = stpool.tile([128, DBLK, R * R], F32, tag="stag", name=f"st{blk}")
            nc.sync.dma_start(out=stag, in_=x_r[:, blk * DBLK:(blk + 1) * DBLK, :])
            xb = xpool.tile([128, DBLK, RP, RP], F32R, tag="xb", name=f"xb{blk}")
            nc.vector.tensor_copy(out=xb[:, :, 0, :], in_=zview)
            nc.vector.tensor_copy(out=xb[:, :, RP - 1, :], in_=zview)
            nc.vector.tensor_copy(out=xb[:, :, :, 0], in_=zview)
            nc.vector.tensor_copy(out=xb[:, :, :, RP - 1], in_=zview)
            stv = stag.rearrange("p d (h w) -> p d h w", w=R)
            for si in range(DBLK):
                nc.vector.tensor_copy(out=xb[:, si, 1:R + 1, 1:R + 1],
                                      in_=stv[:, si, :, :])
            xblocks[blk] = xb

        load_block(0)
        if NBLK > 1:
            load_block(1)

        next_blk = 2
        for g in range(R // GD):
            d0 = g * GD
            # prefetch: ensure block containing d0+GD+1 is loaded
            while next_blk < NBLK and (d0 + GD) // DBLK + 1 >= next_blk:
                load_block(next_blk)
                next_blk += 1

            ds = list(range(d0, d0 + GD))
            # per-d valid kd set and first/last tap bookkeeping
            valid = {d: [kd for kd in range(3) if 0 <= d + kd - 1 < R] for d in ds}
            first_t = {d: min(v) * 9 for d, v in valid.items()}
            last_t = {d: max(v) * 9 + 8 for d, v in valid.items()}

            ps = {(dd, hh): pspool.tile([128, 512], F32, tag=f"ps{dd}{hh}",
                                        name=f"ps{d0}_{dd}{hh}")
                  for dd in range(GD) for hh in range(2)}

            for kd in range(3):
                for kh in range(3):
                    for kw in range(3):
                        t = kd * 9 + kh * 3 + kw
                        lhs = w1r[:, t * 128:(t + 1) * 128]
                        for dd, d in enumerate(ds):
                            s = d + kd - 1
                            if not (0 <= s < R):
                                continue
                            xb = xblocks[s // DBLK]
                            si = s % DBLK
                            for hh in range(2):
                                rhs = xb[:, si, 16 * hh + kh: 16 * hh + kh + 16,
                                         kw:kw + 32]
                                nc.tensor.matmul(
                                    ps[(dd, hh)], lhsT=lhs, rhs=rhs,
                                    start=(t == first_t[d]), stop=(t == last_t[d]))

            for dd, d in enumerate(ds):
                ob = obpool.tile([128, 1024], F32, tag="ob", name=f"ob{d}")
                for hh in range(2):
                    nc.scalar.activation(out=ob[:, hh * 512:(hh + 1) * 512],
                                         in_=ps[(dd, hh)], func=AF.Prelu,
                                         bias=bias_g, scale=scale_sb, alpha=SLOPE)
                nc.sync.dma_start(out=out_d[:, d * 1024:(d + 1) * 1024], in_=ob)

    nc.compile()
    return nc


def kernel(**inputs):
    x = np.ascontiguousarray(np.asarray(inputs["x"], dtype=np.float32))
    w = np.ascontiguousarray(np.asarray(inputs["w"], dtype=np.float32))
    weight = np.ascontiguousarray(np.asarray(inputs["weight"], dtype=np.float32))
    aw = np.ascontiguousarray(np.asarray(inputs["affine_weight"], dtype=np.float32))
    ab = np.ascontiguousarray(np.asarray(inputs["affine_bias"], dtype=np.float32))
    bias = np.ascontiguousarray(np.asarray(inputs["bias"], dtype=np.float32))

    if "nc" not in _cache:
        _cache["nc"] = _build()
    nc = _cache["nc"]

    wt2 = weight.reshape(COUT, CIN * NTAPS)
    in_maps = [
        {
            "x": x[b].reshape(CIN, R * R * R),
            "wvec": w[b],
            "weight": wt2,
            "aw": aw,
            "ab": ab,
            "bias": bias,
        }
        for b in range(B)
    ]
    res = run_bass_kernel_spmd(nc, in_maps, list(range(NCORES)))
    out = np.stack([res.results[b]["out"].reshape(COUT, R, R, R) for b in range(B)])
    return out.astype(np.float32)


def run_traced(**inputs):
    """Like kernel(), but also returns the profiled HW exec time in ns."""
    x = np.asarray(inputs["x"], dtype=np.float32)
    w = np.asarray(inputs["w"], dtype=np.float32)
    weight = np.asarray(inputs["weight"], dtype=np.float32)
    aw = np.asarray(inputs["affine_weight"], dtype=np.float32)
    ab = np.asarray(inputs["affine_bias"], dtype=np.float32)
    bias = np.asarray(inputs["bias"], dtype=np.float32)
    if "nc" not in _cache:
        _cache["nc"] = _build()
    nc = _cache["nc"]
    wt2 = weight.reshape(COUT, CIN * NTAPS)
    in_maps = [
        {"x": x[b].reshape(CIN, R * R * R), "wvec": w[b], "weight": wt2,
         "aw": aw, "ab": ab, "bias": bias}
        for b in range(B)
    ]
    res = run_bass_kernel_spmd(nc, in_maps, list(range(NCORES)), trace=True)
    out = np.stack([res.results[b]["out"].reshape(COUT, R, R, R) for b in range(B)])
    return out.astype(np.float32), res.exec_time_ns, res



# revision 3
# speedup vs baseline: 1.0705x; 1.0705x over previous
"""Trainium2 Bass kernel for nn_Conv3DSynthesisLayer.

Computes, per sample b (one NeuronCore each, data-parallel over batch B=8):
  styles = w[b] @ (affine_weight / sqrt(512)).T + affine_bias        [Cin]
  wmod   = weight * styles[None,:,None..] ; demod by rsqrt(sumsq)    [Cout,Cin,3,3,3]
  out    = lrelu(conv3d(x[b], wmod, pad=1) + bias) * sqrt(2)         [Cout,32,32,32]

Implementation notes (v2, bf16):
  - Conv is 27 shifted bf16 matmuls (K=Cin=128 on partitions) accumulated in
    PSUM per output d-slice, over an h/w zero-padded x laid out per-slice as
    [128, 34 rows x 64-elem pitch] in SBUF (pitch 64 keeps row starts 128B
    aligned).  D-boundary taps are skipped (no d padding).
  - bf16 matmuls issue at ~228ns/512 rows vs fp32r's ~241ns (hw-measured);
    rounding error lands at rel ~2.6e-3, far inside the 2e-2 gate.
  - Modulated bf16 weights are built in one DVE op per tap straight from the
    PE-transposed PSUM tile; the demod sum-of-squares uses Square on the
    Scalar engine + a styles^2 ones-matmul, freeing the DVE for x placement.
  - Startup is reordered so x block placement and the styles chain run before
    the per-tap weight build; the conv starts ~9us in instead of ~35us.
  - Demodulation and the lrelu epilogue stay fused in one ScalarE Prelu per
    PSUM bank: out = prelu(psum*scale + bias*g).
"""
import sys

sys.path.insert(0, "/opt/trn_rl_repo")

import numpy as np
from contextlib import ExitStack

import concourse.mybir as mybir
import concourse.tile as tile
from concourse import bacc
from concourse.masks import make_identity
from concourse import bass_utils as _bass_utils
from concourse.bass_utils import run_bass_kernel_spmd

# Walrus's ldw-opt pass rejects the explicit InstLdweights that bf16 matmuls
# emit ("not compatible with LDW optimization"), so it must stay off; the
# per-matmul bf16 weight loads pipeline under the previous matmul for free
# (hw-measured: alternating-stationary bf16 runs at the same 216ns/512rows).
_LDW_OPT = False
if not getattr(_bass_utils, "_ldw_opt_patched", False):
    _orig_run_command = _bass_utils.run_command

    def _run_command_ldw(argv, **kw):
        if _LDW_OPT and isinstance(argv, (list, tuple)):
            argv = ["--enable-ldw-opt=true" if a == "--enable-ldw-opt=false" else a
                    for a in argv]
        return _orig_run_command(argv, **kw)

    _bass_utils.run_command = _run_command_ldw
    _bass_utils._ldw_opt_patched = True

F32 = mybir.dt.float32
BF16 = mybir.dt.bfloat16
AF = mybir.ActivationFunctionType

B, CIN, COUT, R = 8, 128, 128, 32
W_DIM = 512
NTAPS = 27
RP = R + 2   # 34: h/w padded extent
PW = 64      # row pitch (elems) so bf16 rows start 128B-aligned
GAIN = float(np.sqrt(2.0).astype(np.float32))
SLOPE = 0.2
EPS = 1e-8
DBLK = 4     # d-slices per x block
NBLK = R // DBLK
GD = 2       # d-slices per psum group (taps-outer)
NCORES = 8

_cache = {}


def _build():
    nc = bacc.Bacc("TRN2", target_bir_lowering=False, debug=False, num_devices=NCORES)
    x_d = nc.dram_tensor("x", [CIN, R * R * R], F32, kind="ExternalInput").ap()
    wv_d = nc.dram_tensor("wvec", [W_DIM], F32, kind="ExternalInput").ap()
    wt_d = nc.dram_tensor("weight", [COUT, CIN * NTAPS], F32, kind="ExternalInput").ap()
    aw_d = nc.dram_tensor("aw", [CIN, W_DIM], F32, kind="ExternalInput").ap()
    ab_d = nc.dram_tensor("ab", [CIN], F32, kind="ExternalInput").ap()
    bs_d = nc.dram_tensor("bias", [COUT], F32, kind="ExternalInput").ap()
    out_d = nc.dram_tensor("out", [COUT, R * R * R], F32, kind="ExternalOutput").ap()

    ctx = ExitStack()
    with ctx:
        tc = ctx.enter_context(tile.TileContext(nc))
        singles = ctx.enter_context(tc.tile_pool(name="singles", bufs=1))
        xpool = ctx.enter_context(tc.tile_pool(name="xpool", bufs=4))
        stpool = ctx.enter_context(tc.tile_pool(name="stpool", bufs=2))
        sqpool = ctx.enter_context(tc.tile_pool(name="sqpool", bufs=3))
        obpool = ctx.enter_context(tc.tile_pool(name="obpool", bufs=4))

        # ---- param DMAs up front ----
        aw_sb = singles.tile([128, W_DIM], F32)
        nc.sync.dma_start(out=aw_sb, in_=aw_d)
        wv_sb = singles.tile([128, 4], F32)
        nc.sync.dma_start(out=wv_sb, in_=wv_d.rearrange("(c k) -> k c", k=128))
        ab_sb = singles.tile([128, 1], F32)
        nc.sync.dma_start(out=ab_sb, in_=ab_d.rearrange("(p one) -> p one", one=1))
        bs_sb = singles.tile([128, 1], F32)
        nc.sync.dma_start(out=bs_sb, in_=bs_d.rearrange("(p one) -> p one", one=1))
        wnat = singles.tile([128, CIN * NTAPS], F32)
        nc.sync.dma_start(out=wnat, in_=wt_d)

        # ---- x staging / padded-block machinery ----
        zrow = singles.tile([128, DBLK * PW], F32)
        nc.vector.memset(zrow, 0.0)
        zview = zrow.rearrange("p (d e) -> p d e", e=PW)

        x_r = x_d.rearrange("p (d hw) -> p d hw", hw=R * R)
        xblocks = [None] * NBLK

        def stage_block(blk):
            stag = stpool.tile([128, DBLK, R * R], F32, tag="stag", name=f"st{blk}")
            nc.sync.dma_start(out=stag, in_=x_r[:, blk * DBLK:(blk + 1) * DBLK, :])
            return stag

        def place_block(blk, stag):
            xb = xpool.tile([128, DBLK, RP, PW], BF16, tag="xb", name=f"xb{blk}")
            nc.vector.tensor_copy(out=xb[:, :, 0, 0:RP], in_=zview[:, :, 0:RP])
            nc.vector.tensor_copy(out=xb[:, :, RP - 1, 0:RP], in_=zview[:, :, 0:RP])
            nc.vector.tensor_copy(out=xb[:, :, :, 0], in_=zview[:, :, 0:RP])
            nc.vector.tensor_copy(out=xb[:, :, :, RP - 1], in_=zview[:, :, 0:RP])
            stv = stag.rearrange("p d (h w) -> p d h w", w=R)
            for si in range(DBLK):
                nc.vector.tensor_copy(out=xb[:, si, 1:R + 1, 1:R + 1],
                                      in_=stv[:, si, :, :])
            xblocks[blk] = xb

        st0 = stage_block(0)
        st1 = stage_block(1)

        # ---- phase A: styles, modulated bf16 weights, demod scale ----
        with tc.tile_pool(name="ps_a", bufs=2, space="PSUM") as ps_a:
            ident = singles.tile([128, 128], F32)
            make_identity(nc, ident)

            # affine_weight.T, chunked over the 512-dim
            awt = singles.tile([128, W_DIM], F32)
            for c in range(4):
                paw = ps_a.tile([128, 128], F32, tag="paw", name=f"paw{c}")
                nc.tensor.transpose(paw, aw_sb[:, c * 128:(c + 1) * 128], ident)
                nc.vector.tensor_copy(out=awt[:, c * 128:(c + 1) * 128], in_=paw)

            # styles[ci] = sum_k aw[ci,k] w[k] / sqrt(512) + ab[ci]
            ps_sty = ps_a.tile([128, 1], F32, tag="ps_sty")
            for c in range(4):
                nc.tensor.matmul(ps_sty, lhsT=awt[:, c * 128:(c + 1) * 128],
                                 rhs=wv_sb[:, c:c + 1], start=(c == 0), stop=(c == 3))
            styles = singles.tile([128, 1], F32)
            nc.scalar.activation(out=styles, in_=ps_sty, func=AF.Identity,
                                 bias=ab_sb, scale=1.0 / float(np.sqrt(W_DIM)))
            styles2 = singles.tile([128, 1], F32)
            nc.vector.tensor_mul(out=styles2, in0=styles, in1=styles)

            # x blocks 0/1 placed before the weight chain so conv can start early
            place_block(0, st0)
            place_block(1, st1)

            # per-tap: transpose [co,ci]->[ci,co]; modulate to bf16 (DVE);
            # square on ScalarE; accumulate sumsq via styles^2-matmul (PE).
            wnat_t = wnat.rearrange("p (ci t) -> p t ci", t=NTAPS)
            w1b = singles.tile([128, NTAPS * 128], BF16)
            ps_dm = ps_a.tile([128, 1], F32, tag="ps_dm")
            for t in range(NTAPS):
                pw = ps_a.tile([128, 128], F32, tag="paw", name=f"pw{t}")
                nc.tensor.transpose(pw, wnat_t[:, t, :], ident)
                nc.vector.tensor_scalar_mul(out=w1b[:, t * 128:(t + 1) * 128],
                                            in0=pw, scalar1=styles)
                sq = sqpool.tile([128, 128], F32, tag="sq", name=f"sq{t}")
                nc.scalar.activation(out=sq, in_=pw, func=AF.Square)
                nc.tensor.matmul(ps_dm, lhsT=sq, rhs=styles2,
                                 start=(t == 0), stop=(t == NTAPS - 1))

            # scale[co] = GAIN * rsqrt(sumsq+EPS) = 1/sqrt(sumsq/G^2 + EPS/G^2)
            eps_sb = singles.tile([128, 1], F32)
            nc.vector.memset(eps_sb, EPS / (GAIN * GAIN))
            sc_tmp = singles.tile([128, 1], F32)
            nc.scalar.activation(out=sc_tmp, in_=ps_dm, func=AF.Sqrt,
                                 bias=eps_sb, scale=1.0 / (GAIN * GAIN))
            scale_sb = singles.tile([128, 1], F32)
            nc.vector.reciprocal(out=scale_sb, in_=sc_tmp)
            bias_g = singles.tile([128, 1], F32)
            nc.scalar.mul(out=bias_g, in_=bs_sb, mul=GAIN)

        pspool = ctx.enter_context(tc.tile_pool(name="pspool", bufs=2, space="PSUM"))

        # ---- phase B: the conv ----
        next_blk = 2
        for g in range(R // GD):
            d0 = g * GD
            while next_blk < NBLK and (d0 + GD) // DBLK + 1 >= next_blk:
                place_block(next_blk, stage_block(next_blk))
                next_blk += 1

            ds = list(range(d0, d0 + GD))
            valid = {d: [kd for kd in range(3) if 0 <= d + kd - 1 < R] for d in ds}
            first_t = {d: min(v) * 9 for d, v in valid.items()}
            last_t = {d: max(v) * 9 + 8 for d, v in valid.items()}

            ps = {(dd, hh): pspool.tile([128, 512], F32, tag=f"ps{dd}{hh}",
                                        name=f"ps{d0}_{dd}{hh}")
                  for dd in range(GD) for hh in range(2)}

            for kd in range(3):
                for kh in range(3):
                    for kw in range(3):
                        t = kd * 9 + kh * 3 + kw
                        lhs = w1b[:, t * 128:(t + 1) * 128]
                        for dd, d in enumerate(ds):
                            s = d + kd - 1
                            if not (0 <= s < R):
                                continue
                            xb = xblocks[s // DBLK]
                            si = s % DBLK
                            for hh in range(2):
                                rhs = xb[:, si, 16 * hh + kh: 16 * hh + kh + 16,
                                         kw:kw + 32]
                                nc.tensor.matmul(
                                    ps[(dd, hh)], lhsT=lhs, rhs=rhs,
                                    start=(t == first_t[d]), stop=(t == last_t[d]))

            for dd, d in enumerate(ds):
                ob = obpool.tile([128, 1024], F32, tag="ob", name=f"ob{d}")
                for hh in range(2):
                    nc.scalar.activation(out=ob[:, hh * 512:(hh + 1) * 512],
                                         in_=ps[(dd, hh)], func=AF.Prelu,
                                         bias=bias_g, scale=scale_sb, alpha=SLOPE)
                nc.sync.dma_start(out=out_d[:, d * 1024:(d + 1) * 1024], in_=ob)

    nc.compile()
    return nc


def kernel(**inputs):
    x = np.ascontiguousarray(np.asarray(inputs["x"], dtype=np.float32))
    w = np.ascontiguousarray(np.asarray(inputs["w"], dtype=np.float32))
    weight = np.ascontiguousarray(np.asarray(inputs["weight"], dtype=np.float32))
    aw = np.ascontiguousarray(np.asarray(inputs["affine_weight"], dtype=np.float32))
    ab = np.ascontiguousarray(np.asarray(inputs["affine_bias"], dtype=np.float32))
    bias = np.ascontiguousarray(np.asarray(inputs["bias"], dtype=np.float32))

    if "nc" not in _cache:
        _cache["nc"] = _build()
    nc = _cache["nc"]

    wt2 = weight.reshape(COUT, CIN * NTAPS)
    in_maps = [
        {
            "x": x[b].reshape(CIN, R * R * R),
            "wvec": w[b],
            "weight": wt2,
            "aw": aw,
            "ab": ab,
            "bias": bias,
        }
        for b in range(B)
    ]
    res = run_bass_kernel_spmd(nc, in_maps, list(range(NCORES)))
    out = np.stack([res.results[b]["out"].reshape(COUT, R, R, R) for b in range(B)])
    return out.astype(np.float32)


def run_traced(**inputs):
    """Like kernel(), but also returns the profiled HW exec time in ns."""
    x = np.asarray(inputs["x"], dtype=np.float32)
    w = np.asarray(inputs["w"], dtype=np.float32)
    weight = np.asarray(inputs["weight"], dtype=np.float32)
    aw = np.asarray(inputs["affine_weight"], dtype=np.float32)
    ab = np.asarray(inputs["affine_bias"], dtype=np.float32)
    bias = np.asarray(inputs["bias"], dtype=np.float32)
    if "nc" not in _cache:
        _cache["nc"] = _build()
    nc = _cache["nc"]
    wt2 = weight.reshape(COUT, CIN * NTAPS)
    in_maps = [
        {"x": x[b].reshape(CIN, R * R * R), "wvec": w[b], "weight": wt2,
         "aw": aw, "ab": ab, "bias": bias}
        for b in range(B)
    ]
    res = run_bass_kernel_spmd(nc, in_maps, list(range(NCORES)), trace=True)
    out = np.stack([res.results[b]["out"].reshape(COUT, R, R, R) for b in range(B)])
    return out.astype(np.float32), res.exec_time_ns, res
